# revision 83
# baseline (speedup 1.0000x reference)
"""BigramAttn Trainium2 kernel (8-core SPMD, raw Bass) — fp16 pipeline.

Reference computation (per batch b):
  e[0]   = sum_k enc[0,k] * h[k]
  e[s]   = sum_k (enc[s-1,:] @ M)[k] * h[k] * enc[s,k]          (s >= 1)
  e[s]  += sum_{k<3} (h @ affect)[k] * emb[s,k]
  out    = softmax(e)                                            # over s

Sharding: data-parallel over batch B=32 across 8 cores (4 batches/core).

fp16 data path (measured end-to-end rel err ~4e-3 vs the 2e-2 gate):
enc/M/emb/ha ship as fp16 (halves HBM traffic vs fp32; total ~17.5MB/core),
all matmuls are fp16 in / fp32 psum accumulate. h stays fp32 and is folded
on device (scalar_tensor_tensor per-partition scalar), so M is a single
shared 512KB load instead of per-batch M*diag(h) copies.

Per chunk-step (b, c) over a 513-wide enc window (1-col halo, host-packed
contiguous so each step is ONE 525KB DMA):
  PE:   A_kt[k,t] = sum_j M^T enc_prev      (16 fp16 MMs -> ps_a, 4 banks)
  ACT:  Y_01 = fp16(h_01 * A_01)            (copy+scale psum->sbuf)
  DVE:  P01  = Y01 * enc_01                 (fp16 TT, 2x mode)
  DVE:  P2,P3 = (A_kt*h_kt)*enc_kt          (stt, psum 1x; GPSIMD can't
                                             touch PSUM on trn2)
  POOL: Q = (P0+P1) + (P2+P3)               (fp16 add tree, SBUF only)
  PE:   e = ones^T Q + ha^T emb_c           (2 accumulating MMs -> ps_e)
  DVE:  nm_c = -max(e)                      (per-chunk max)
  ACT:  ex_c = exp(e + nm_c), sm_c = sum    (flash-style, overlapped)
Tail per batch: m_b = max_c mx_c; ed_c = exp(mx_c-m_b); Z_b = sum ed_c*sm_c;
alpha_c = ed_c/Z_b; out = ex_c * alpha_c (rescale split DVE/ACT/POOL).

This walrus build accepts exactly ONE semaphore wait per instruction, so the
kernel is raw Bass: per-engine programs, counting semaphores, standalone
waits. DMA completions may reorder across transfers, so chunk DMAs chain on
per-lane semaphores.
"""

import functools

import numpy as np

import concourse.bass as bass
from concourse import mybir
from concourse.bass_utils import run_bass_kernel_spmd

S, B, H = 4096, 32, 512
NCORES = 8
BC = B // NCORES          # batches per core = 4
NK = H // 128             # h-chunks = 4
CH = 512                  # s-chunk width
CW = CH + 1               # packed chunk block width (1-col halo)
NCH = S // CH             # s-chunks per batch = 8
NBC = BC * NCH            # chunk-steps per core = 32
NSLOT = 4                 # enc chunk tiles in flight
NLANE = 4                 # DMA completion-ordering lanes
NEB = 3                   # ps_e rotation depth
SETUP_DMAS = 9            # m, ht, ha, ones16, ones32, 4x emb

F32 = mybir.dt.float32
F16 = mybir.dt.float16


@functools.lru_cache(maxsize=1)
def _build():
    nc = bass.Bass("TRN2", target_bir_lowering=False, debug=False)

    enc_c = nc.dram_tensor("enc_c", [NBC, 128, NK * CW], F16,
                           kind="ExternalInput").ap()
    m_p = nc.dram_tensor("m_p", [128, NK * H], F16, kind="ExternalInput").ap()
    h_t = nc.dram_tensor("h_t", [128, BC * NK], F32, kind="ExternalInput").ap()
    one_h = nc.dram_tensor("one_h", [128, 1], F16, kind="ExternalInput").ap()
    one_f = nc.dram_tensor("one_f", [128, 1], F32, kind="ExternalInput").ap()
    emb_a = nc.dram_tensor("emb_a", [3 * BC, S], F16, kind="ExternalInput").ap()
    out = nc.dram_tensor("out", [BC, S], F32, kind="ExternalOutput").ap()

    # SBUF
    enc_sb = [nc.alloc_sbuf_tensor(f"enc{i}", [128, NK * CW], F16).ap()
              for i in range(NSLOT)]
    m_sb = nc.alloc_sbuf_tensor("m", [128, NK * H], F16).ap()
    ht_sb = nc.alloc_sbuf_tensor("ht", [128, BC * NK], F32).ap()
    oh_sb = nc.alloc_sbuf_tensor("oh", [128, 1], F16).ap()
    of_sb = nc.alloc_sbuf_tensor("of", [128, 1], F32).ap()
    emb_sb = [nc.alloc_sbuf_tensor(f"emb{b}", [3, S], F16).ap()
              for b in range(BC)]
    y_sb = [nc.alloc_sbuf_tensor(f"y{i}", [128, 2 * CH], F16).ap()
            for i in range(2)]
    p_sb = [nc.alloc_sbuf_tensor(f"p{i}", [128, NK * CH], F16).ap()
            for i in range(2)]
    q_sb = [nc.alloc_sbuf_tensor(f"q{i}", [128, 3 * CH], F16).ap()
            for i in range(3)]
    ex_w = nc.alloc_sbuf_tensor("ex_w", [128, S], F32).ap()
    aw = nc.alloc_sbuf_tensor("aw", [128, NCH], F32).ap()
    ex4 = nc.alloc_sbuf_tensor("ex4", [BC, S], F32).ap()
    o4 = nc.alloc_sbuf_tensor("o4", [BC, S], F32).ap()
    nm0 = nc.alloc_sbuf_tensor("nm0", [1, NBC], F32).ap()   # -chunk max
    sm0 = nc.alloc_sbuf_tensor("sm0", [1, NBC], F32).ap()   # chunk expsum
    mnb = nc.alloc_sbuf_tensor("mnb", [1, BC], F32).ap()    # min_c nm = -m_b
    ed0 = nc.alloc_sbuf_tensor("ed0", [1, NBC], F32).ap()   # exp(mx_c-m_b)
    w0 = nc.alloc_sbuf_tensor("w0", [1, NBC], F32).ap()
    zb = nc.alloc_sbuf_tensor("zb", [1, BC], F32).ap()
    rz = nc.alloc_sbuf_tensor("rz", [1, BC], F32).ap()
    al4 = nc.alloc_sbuf_tensor("al4", [BC, NCH], F32).ap()

    # PSUM: 4 banks A + 3 banks e = 7 of 8
    ps_a = nc.alloc_psum_tensor("psA", [128, NK * CH], F32).ap()
    ps_e = [nc.alloc_psum_tensor(f"psE{i}", [128, CH], F32).ap()
            for i in range(NEB)]

    dma_set = nc.alloc_semaphore("dma_set")  # oh, ha, emb (reduce deps)
    dma_m = nc.alloc_semaphore("dma_m")      # m_sb (PE main dep)
    dma_h = nc.alloc_semaphore("dma_h")      # ht, of (ACT/DVE deps)
    dma_ln = [nc.alloc_semaphore(f"dma_ln{k}") for k in range(NLANE)]
    dma_g = nc.alloc_semaphore("dma_g")
    dma_out = nc.alloc_semaphore("dma_out")
    pe_mm = nc.alloc_semaphore("pe_mm")      # +1 per kt MM-group (4/step)
    pe_red = nc.alloc_semaphore("pe_red")    # +1 per step e-reduce
    act_y = nc.alloc_semaphore("act_y")      # +1 per Y half (2/step)
    act_exp = nc.alloc_semaphore("act_exp")  # +1 per chunk exp
    act_ed = nc.alloc_semaphore("act_ed")    # +1 per batch ed
    act_fin = nc.alloc_semaphore("act_fin")
    dve_p = nc.alloc_semaphore("dve_p")      # +1 per step P01 mul
    dve_qf = nc.alloc_semaphore("dve_qf")    # +1 per step Q final (DVE)
    dve_s = nc.alloc_semaphore("dve_s")      # +1 per stt (2/step)
    dve_sd = nc.alloc_semaphore("dve_sd")    # DVE col-0 seed self-sync
    dve_mx = nc.alloc_semaphore("dve_mx")    # +1 per chunk max
    dve_tl = nc.alloc_semaphore("dve_tl")    # +1 per batch mnb
    dve_tc = nc.alloc_semaphore("dve_tc")    # tail chain self-sync counter
    dve_al = nc.alloc_semaphore("dve_al")    # +1 alphas ready
    dve_fin = nc.alloc_semaphore("dve_fin")
    act_sd = nc.alloc_semaphore("act_sd")    # ACT col-0 seed self-sync
    pool_t2 = nc.alloc_semaphore("pool_t2")  # +3 per step (Q1, Q2, ew)

    EXP = mybir.ActivationFunctionType.Exp

    with nc.Block() as blk:
        # --- SP: all DMAs ---
        @blk.sync
        def _(sync):
            # first chunk first: it gates the first matmul (m/ht/of go out
            # in parallel on ACT's HWDGE queue). Setup DMAs use
            # per-consumer-group semaphores (completions can reorder, so
            # partial-sum thresholds on one semaphore would be ambiguous).
            sync.dma_start(enc_sb[0][:], enc_c[0]).then_inc(dma_ln[0], 16)
            setup = [
                (oh_sb[:], one_h[:]),
                *[(emb_sb[b][:], emb_a[3 * b:3 * b + 3, :])
                  for b in range(BC)],
            ]
            for dst, src in setup:
                sync.dma_start(dst, src).then_inc(dma_set, 16)
            for bc in range(1, NBC):
                if bc >= NSLOT:
                    p = bc - NSLOT
                    sync.wait_ge(pe_mm, 4 * p + 4)
                    sync.wait_ge(dve_p, p + 1)
                    sync.wait_ge(dve_s, 2 * p + 2)
                if bc >= NLANE:
                    sync.wait_ge(dma_ln[bc % NLANE], 16 * (bc // NLANE))
                sync.dma_start(enc_sb[bc % NSLOT][:], enc_c[bc]) \
                    .then_inc(dma_ln[bc % NLANE], 16)
            # gather ex_w batch rows chunk-by-chunk as their exps complete
            for cc in range(NCH):
                sync.wait_ge(act_exp, 3 * NCH + cc + 1)  # exp(24+cc) done
                sync.dma_start(ex4[:, cc * CH:(cc + 1) * CH],
                               ex_w[0:128:32, cc * CH:(cc + 1) * CH]) \
                    .then_inc(dma_g, 16)
            sync.wait_ge(dve_al, 1)
            sync.dma_start(al4[:], aw[0:128:32, :]).then_inc(dma_g, 16)
            # out in pieces as rescale chunks land (ACT issues its own piece)
            sync.wait_ge(dve_fin, 1)
            sync.dma_start(out[:, 0:2 * CH], o4[:, 0:2 * CH]) \
                .then_inc(dma_out, 16)
            sync.wait_ge(dve_fin, 2)
            sync.dma_start(out[:, 2 * CH:5 * CH], o4[:, 2 * CH:5 * CH]) \
                .then_inc(dma_out, 16)
            sync.wait_ge(dma_out, 48)

        # --- PE ---
        @blk.tensor
        def _(tensor):
            def reduce(j):
                r = j % NEB
                if j == 0:
                    tensor.wait_ge(dma_set, 80)  # oh, emb
                tensor.wait_ge(dve_qf, j + 1)
                if j >= NEB:
                    tensor.wait_ge(act_exp, j - NEB + 1)  # WAR ps_e[r]
                nc.tensor.matmul(
                    ps_e[r][0:1, 0:CH], oh_sb[:, 0:1],
                    q_sb[j % 3][:, 2 * CH:3 * CH],
                    start=True, stop=True).then_inc(pe_red, 1)

            tensor.wait_ge(dma_m, 16)            # m_sb only
            for bc in range(NBC):
                slot = bc % NSLOT
                tensor.wait_ge(dma_ln[bc % NLANE], 16 * (bc // NLANE + 1))
                for kt in range(NK):
                    # per-bank WAR: wait only for the drain of THIS bank from
                    # the previous step, so late stt's don't stall early MMs
                    if bc >= 1:
                        if kt == 0:
                            tensor.wait_ge(act_y, 2 * bc)       # Y of bc-1
                        elif kt == 2:
                            tensor.wait_ge(dve_s, 2 * bc - 1)   # stt P2
                        elif kt == 3:
                            tensor.wait_ge(dve_s, 2 * bc)       # stt P3
                    for j in range(NK):
                        mm = nc.tensor.matmul(
                            ps_a[:, kt * CH:(kt + 1) * CH],
                            m_sb[:, j * H + kt * 128:j * H + (kt + 1) * 128],
                            enc_sb[slot][:, j * CW:j * CW + CH],
                            start=(j == 0), stop=(j == NK - 1))
                    mm.then_inc(pe_mm, 1)
                # lag-2 reduce: gives the P->Q chain a full extra step of
                # slack so the reduce never stalls the PE
                if bc >= 2:
                    reduce(bc - 2)
            reduce(NBC - 2)
            reduce(NBC - 1)

        # --- ACT: Y copies (h-fold, psum->sbuf fp16), chunk exp, batch ed ---
        @blk.scalar
        def _(scalar):
            def exp_op(j):
                b, c, r = j // NCH, j % NCH, j % NEB
                scalar.wait_ge(pe_red, j + 1)
                scalar.wait_ge(dve_mx, j + 1)
                nc.scalar.activation(
                    ex_w[32 * b:32 * b + 1, c * CH:(c + 1) * CH],
                    ps_e[r][0:1, 0:CH],
                    EXP, bias=nm0[0:1, j:j + 1],
                    accum_out=sm0[0:1, j:j + 1]).then_inc(act_exp, 1)

            def ed_op(b):
                scalar.wait_ge(dve_tl, b + 1)
                nc.scalar.activation(
                    ed0[0:1, NCH * b:NCH * (b + 1)],
                    nm0[0:1, NCH * b:NCH * (b + 1)],
                    EXP, bias=mnb[0:1, b:b + 1], scale=-1.0) \
                    .then_inc(act_ed, 1)

            # issue m/ht/of on ACT's own HWDGE queue, parallel to SP's chunk0
            scalar.dma_start(m_sb[:], m_p[:]).then_inc(dma_m, 16)
            scalar.dma_start(ht_sb[:], h_t[:]).then_inc(dma_h, 16)
            scalar.dma_start(of_sb[:], one_f[:]).then_inc(dma_h, 16)
            scalar.wait_ge(dma_h, 32)            # ht, of
            n_sd = 0
            for bc in range(NBC):
                b, c, par = bc // NCH, bc % NCH, bc % 2
                scalar.wait_ge(pe_mm, 4 * bc + 1)
                if bc >= 2:
                    scalar.wait_ge(dve_p, bc - 1)    # WAR y_sb[par]
                if c == 0:
                    nc.scalar.copy(ps_a[:, 0:1], of_sb[:]) \
                        .then_inc(act_sd, 1)
                    n_sd += 1
                    scalar.wait_ge(act_sd, n_sd)
                nc.scalar.mul(y_sb[par][:, 0:CH], ps_a[:, 0:CH],
                              ht_sb[:, NK * b:NK * b + 1]).then_inc(act_y, 1)
                scalar.wait_ge(pe_mm, 4 * bc + 2)
                if c == 0:
                    nc.scalar.copy(ps_a[:, CH:CH + 1], of_sb[:]) \
                        .then_inc(act_sd, 1)
                    n_sd += 1
                    scalar.wait_ge(act_sd, n_sd)
                nc.scalar.mul(y_sb[par][:, CH:2 * CH], ps_a[:, CH:2 * CH],
                              ht_sb[:, NK * b + 1:NK * b + 2]) \
                    .then_inc(act_y, 1)
                # lag-3 exp: pe_red/dve_mx for step j land during step j+2,
                # so an earlier exp would stall ACT and delay the next Y pair
                if bc >= 3:
                    exp_op(bc - 3)
                if bc % NCH == 3 and bc > NCH:
                    ed_op(bc // NCH - 1)
            exp_op(NBC - 3)
            exp_op(NBC - 2)
            exp_op(NBC - 1)
            ed_op(BC - 1)
            # rescale chunks 5-7, then ship that piece on ACT's own queue
            scalar.wait_ge(dma_g, 16 * (NCH + 1))
            for cc in range(5, NCH):
                nc.scalar.mul(o4[:, cc * CH:(cc + 1) * CH],
                              ex4[:, cc * CH:(cc + 1) * CH],
                              al4[0:BC, cc:cc + 1]).then_inc(act_fin, 1)
            scalar.wait_ge(act_fin, 3)           # own o4 writes acked
            scalar.dma_start(out[:, 5 * CH:S], o4[:, 5 * CH:S]) \
                .then_inc(dma_out, 16)

        # --- DVE: P01 mul, Q tree, chunk max, tail combine, rescale ---
        @blk.vector
        def _(vector):
            def mx_op(j):
                r = j % NEB
                vector.wait_ge(pe_red, j + 1)
                nc.vector.tensor_reduce(
                    nm0[0:1, j:j + 1], ps_e[r][0:1, 0:CH],
                    mybir.AxisListType.X, mybir.AluOpType.max,
                    negate=True).then_inc(dve_mx, 1)

            n_tc = 0

            def mnb_op(b):
                vector.wait_ge(dve_mx, NCH * (b + 1))  # own nm0 writes acked
                nc.vector.tensor_reduce(
                    mnb[0:1, b:b + 1], nm0[0:1, NCH * b:NCH * (b + 1)],
                    mybir.AxisListType.X, mybir.AluOpType.min) \
                    .then_inc(dve_tl, 1)

            def wz_op(b):
                nonlocal n_tc
                vector.wait_ge(act_ed, b + 1)
                vector.wait_ge(act_exp, NCH * (b + 1))
                nc.vector.tensor_mul(w0[0:1, NCH * b:NCH * (b + 1)],
                                     ed0[0:1, NCH * b:NCH * (b + 1)],
                                     sm0[0:1, NCH * b:NCH * (b + 1)]) \
                    .then_inc(dve_tc, 1)
                n_tc += 1
                vector.wait_ge(dve_tc, n_tc)
                nc.vector.tensor_reduce(
                    zb[0:1, b:b + 1], w0[0:1, NCH * b:NCH * (b + 1)],
                    mybir.AxisListType.X, mybir.AluOpType.add) \
                    .then_inc(dve_tc, 1)
                n_tc += 1

            n_sd = 0
            vector.wait_ge(dma_h, 32)            # ht, of
            def q_op(j):
                # final Q add (fp16 2x): pool's Q1 + (Q2 incl. affect rows)
                vector.wait_ge(pool_t2, 3 * j + 3)
                if j >= 3:
                    vector.wait_ge(pe_red, j - 2)    # WAR q_sb[j%3] Q slice
                nc.vector.tensor_add(q_sb[j % 3][:, 2 * CH:3 * CH],
                                     q_sb[j % 3][:, 0:CH],
                                     q_sb[j % 3][:, CH:2 * CH]) \
                    .then_inc(dve_qf, 1)

            for bc in range(NBC):
                b, c, par, slot = bc // NCH, bc % NCH, bc % 2, bc % NSLOT
                # P01 = Y01 * E01
                vector.wait_ge(act_y, 2 * bc + 2)
                if bc >= 2:
                    vector.wait_ge(pool_t2, 3 * bc - 4)  # WAR p_sb[par]
                nc.vector.tensor_mul(
                    p_sb[par].rearrange("p (k s) -> p k s", k=NK)[:, 0:2, :],
                    y_sb[par].rearrange("p (k s) -> p k s", k=2)[:, :, :],
                    enc_sb[slot].rearrange("p (k w) -> p k w", k=NK)
                    [:, 0:2, 1:CW]).then_inc(dve_p, 1)
                if bc >= 1:
                    q_op(bc - 1)
                # P2, P3 stt folds (psum)
                for kt in (2, 3):
                    vector.wait_ge(pe_mm, 4 * bc + kt + 1)
                    if c == 0:
                        nc.vector.tensor_copy(
                            ps_a[:, kt * CH:kt * CH + 1], of_sb[:]) \
                            .then_inc(dve_sd, 1)
                        n_sd += 1
                        vector.wait_ge(dve_sd, n_sd)
                    nc.vector.scalar_tensor_tensor(
                        p_sb[par][:, kt * CH:(kt + 1) * CH],
                        ps_a[:, kt * CH:(kt + 1) * CH],
                        ht_sb[:, NK * b + kt:NK * b + kt + 1],
                        enc_sb[slot][:, kt * CW + 1:kt * CW + CW],
                        mybir.AluOpType.mult, mybir.AluOpType.mult) \
                        .then_inc(dve_s, 1)
                if bc >= 2:
                    mx_op(bc - 2)
                if bc % NCH == 2 and bc > NCH:
                    mnb_op(bc // NCH - 1)
                if bc % NCH == 3 and bc > NCH:
                    wz_op(bc // NCH - 1)
            q_op(NBC - 1)
            mx_op(NBC - 2)
            mx_op(NBC - 1)
            mnb_op(BC - 1)
            wz_op(BC - 1)
            vector.wait_ge(dve_tc, n_tc)             # zb writes acked
            nc.vector.reciprocal(rz[0:1, 0:BC], zb[0:1, 0:BC]) \
                .then_inc(dve_tc, 1)
            n_tc += 1
            vector.wait_ge(dve_tc, n_tc)             # rz write acked
            for b in range(BC):
                op = nc.vector.tensor_scalar_mul(
                    aw[32 * b:32 * b + 1, 0:NCH],
                    ed0[0:1, NCH * b:NCH * (b + 1)], rz[0:1, b:b + 1])
            op.then_inc(dve_al, 1)
            # rescale chunks 0-4 (dve_fin: +1 after chunks 0-1, +1 after 2-4)
            vector.wait_ge(dma_g, 16 * (NCH + 1))
            for cc in range(5):
                op = nc.vector.tensor_scalar_mul(
                    o4[:, cc * CH:(cc + 1) * CH],
                    ex4[:, cc * CH:(cc + 1) * CH], al4[0:BC, cc:cc + 1])
                if cc == 1:
                    op.then_inc(dve_fin, 1)
            op.then_inc(dve_fin, 1)

        # --- POOL (gpsimd): P2,P3 stt folds, rescale chunks 6,7 ---
        @blk.gpsimd
        def _(gpsimd):
            gpsimd.wait_ge(dma_set, 80)              # emb (ew-adds)
            for bc in range(NBC):
                b, c, par, qar = bc // NCH, bc % NCH, bc % 2, bc % 3
                if bc >= 3:
                    gpsimd.wait_ge(dve_qf, bc - 2)   # WAR q_sb[qar] Q1/Q2
                gpsimd.wait_ge(dve_p, bc + 1)        # P01 landed
                nc.gpsimd.tensor_add(q_sb[qar][:, 0:CH],
                                     p_sb[par][:, 0:CH],
                                     p_sb[par][:, CH:2 * CH]) \
                    .then_inc(pool_t2, 1)
                gpsimd.wait_ge(dve_s, 2 * bc + 2)    # P2, P3 landed
                nc.gpsimd.tensor_add(q_sb[qar][:, CH:2 * CH],
                                     p_sb[par][:, 2 * CH:3 * CH],
                                     p_sb[par][:, 3 * CH:4 * CH]) \
                    .then_inc(pool_t2, 1)
                # affect term into Q2 rows 0:3 (emb pre-scaled by ha on host)
                gpsimd.wait_ge(pool_t2, 3 * bc + 2)  # own Q2 write acked
                nc.gpsimd.tensor_add(q_sb[qar][0:3, CH:2 * CH],
                                     q_sb[qar][0:3, CH:2 * CH],
                                     emb_sb[b][0:3, c * CH:(c + 1) * CH]) \
                    .then_inc(pool_t2, 1)

    return nc


def _shard_host(hidden, encoder_outputs, embedding, bigram_matrix, affect_matrix):
    """Per-core input maps. Layout/cast prep only (plus tiny h@affect)."""
    h = np.asarray(hidden, dtype=np.float32)[0]              # [B, H]
    enc = np.asarray(encoder_outputs, dtype=np.float32)      # [S, B, H]
    emb = np.asarray(embedding, dtype=np.float32)            # [S, B, 3]
    m = np.asarray(bigram_matrix, dtype=np.float32)
    aff = np.asarray(affect_matrix, dtype=np.float32)        # [H, 3]

    # padded fp16 enc: row 0 is the s=-1 halo for c==0 (value irrelevant;
    # psum col 0 is re-seeded on device)
    encp = np.zeros((S + 1, B, H), dtype=np.float16)
    encp[1:] = enc.astype(np.float16)

    m16 = m.astype(np.float16)
    m_p = np.ascontiguousarray(
        m16.reshape(NK, 128, H).transpose(1, 0, 2).reshape(128, NK * H))
    ha = h @ aff                                             # [B, 3]
    # affect term pre-scaled: emb_w[b, k, s] = ha[b,k] * emb[s, b, k]
    emb16 = np.ascontiguousarray(
        (emb.transpose(1, 2, 0) * ha[:, :, None]).astype(np.float16))
    one_h = np.ones((128, 1), dtype=np.float16)
    one_f = np.ones((128, 1), dtype=np.float32)

    in_maps = []
    for co in range(NCORES):
        b0 = co * BC
        # enc_c[b*NCH+c, p, k*CW+w] = encp[c*CH+w, b0+b, k*128+p]
        blocks = []
        for b in range(b0, b0 + BC):
            v = np.ascontiguousarray(encp[:, b, :])          # [S+1, H]
            w = np.lib.stride_tricks.as_strided(
                v, shape=(NCH, CW, H),
                strides=(CH * v.strides[0], v.strides[0], v.strides[1]))
            t = w.transpose(0, 2, 1).reshape(NCH, NK, 128, CW)
            blocks.append(t.transpose(0, 2, 1, 3).reshape(NCH, 128, NK * CW))
        enc_cc = np.ascontiguousarray(np.concatenate(blocks, axis=0))
        h_sl = h[b0:b0 + BC]                                 # [BC, H]
        ht = np.ascontiguousarray(
            h_sl.reshape(BC, NK, 128).transpose(2, 0, 1).reshape(128, BC * NK))
        in_maps.append({
            "enc_c": enc_cc,
            "m_p": m_p,
            "h_t": ht,
            "one_h": one_h,
            "one_f": one_f,
            "emb_a": emb16[b0:b0 + BC].reshape(3 * BC, S),
        })
    return in_maps


def kernel(hidden, encoder_outputs, embedding, bigram_matrix, affect_matrix,
           _want_results=False, _spmd_kwargs=None):
    nc = _build()
    in_maps = _shard_host(hidden, encoder_outputs, embedding,
                          bigram_matrix, affect_matrix)
    res = run_bass_kernel_spmd(nc, in_maps, core_ids=list(range(NCORES)),
                               **(_spmd_kwargs or {}))
    outp = np.empty((B, 1, S), dtype=np.float32)
    for co in range(NCORES):
        outp[co * BC:(co + 1) * BC, 0, :] = res.results[co]["out"]
    if _want_results:
        return outp, res
    return outp


# revision 89
# speedup vs baseline: 1.3763x; 1.3763x over previous
"""BigramAttn Trainium2 kernel (8-core SPMD, raw Bass) — fp16 pipeline.

Reference computation (per batch b):
  e[0]   = sum_k enc[0,k] * h[k]
  e[s]   = sum_k (enc[s-1,:] @ M)[k] * h[k] * enc[s,k]          (s >= 1)
  e[s]  += sum_{k<3} (h @ affect)[k] * emb[s,k]
  out    = softmax(e)                                            # over s

Sharding: data-parallel over batch B=32 across 8 cores (4 batches/core).

fp16 data path (measured end-to-end rel err ~4e-3 vs the 2e-2 gate):
enc/M/emb/ha ship as fp16 (halves HBM traffic vs fp32; total ~17.5MB/core),
all matmuls are fp16 in / fp32 psum accumulate. h stays fp32 and is folded
on device (scalar_tensor_tensor per-partition scalar), so M is a single
shared 512KB load instead of per-batch M*diag(h) copies.

Per chunk-step (b, c) over a 513-wide enc window (1-col halo, host-packed
contiguous so each step is ONE 525KB DMA):
  PE:   A_kt[k,t] = sum_j M^T enc_prev      (16 fp16 MMs -> ps_a, 4 banks)
  ACT:  Y_01 = fp16(h_01 * A_01)            (copy+scale psum->sbuf)
  DVE:  P01  = Y01 * enc_01                 (fp16 TT, 2x mode)
  DVE:  P2,P3 = (A_kt*h_kt)*enc_kt          (stt, psum 1x; GPSIMD can't
                                             touch PSUM on trn2)
  POOL: Q = (P0+P1) + (P2+P3)               (fp16 add tree, SBUF only)
  PE:   e = ones^T Q + ha^T emb_c           (2 accumulating MMs -> ps_e)
  DVE:  nm_c = -max(e)                      (per-chunk max)
  ACT:  ex_c = exp(e + nm_c), sm_c = sum    (flash-style, overlapped)
Tail per batch: m_b = max_c mx_c; ed_c = exp(mx_c-m_b); Z_b = sum ed_c*sm_c;
alpha_c = ed_c/Z_b; out = ex_c * alpha_c (rescale split DVE/ACT/POOL).

This walrus build accepts exactly ONE semaphore wait per instruction, so the
kernel is raw Bass: per-engine programs, counting semaphores, standalone
waits. DMA completions may reorder across transfers, so chunk DMAs chain on
per-lane semaphores.
"""

import functools

import numpy as np

import concourse.bass as bass
from concourse import mybir
from concourse.bass_utils import run_bass_kernel_spmd

S, B, H = 4096, 32, 512
NCORES = 8
BC = B // NCORES          # batches per core = 4
NK = H // 128             # h-chunks = 4
CH = 512                  # s-chunk width
CW = CH + 1               # packed chunk block width (1-col halo)
NCH = S // CH             # s-chunks per batch = 8
NBC = BC * NCH            # chunk-steps per core = 32
NSLOT = 4                 # enc chunk tiles in flight
NLANE = 4                 # DMA completion-ordering lanes
NEB = 3                   # ps_e rotation depth
SETUP_DMAS = 9            # m, ht, ha, ones16, ones32, 4x emb

F32 = mybir.dt.float32
F16 = mybir.dt.float16


@functools.lru_cache(maxsize=1)
def _build():
    nc = bass.Bass("TRN2", target_bir_lowering=False, debug=False)

    enc_c = nc.dram_tensor("enc_c", [NBC, 128, NK * CW], F16,
                           kind="ExternalInput").ap()
    m_p = nc.dram_tensor("m_p", [128, NK * H], F16, kind="ExternalInput").ap()
    h_t = nc.dram_tensor("h_t", [128, BC * NK], F32, kind="ExternalInput").ap()
    one_h = nc.dram_tensor("one_h", [128, 1], F16, kind="ExternalInput").ap()
    one_f = nc.dram_tensor("one_f", [128, 1], F32, kind="ExternalInput").ap()
    emb_a = nc.dram_tensor("emb_a", [3 * BC, S], F16, kind="ExternalInput").ap()
    out = nc.dram_tensor("out", [BC, S], F32, kind="ExternalOutput").ap()

    # SBUF
    enc_sb = [nc.alloc_sbuf_tensor(f"enc{i}", [128, NK * CW], F16).ap()
              for i in range(NSLOT)]
    m_sb = nc.alloc_sbuf_tensor("m", [128, NK * H], F16).ap()
    ht_sb = nc.alloc_sbuf_tensor("ht", [128, BC * NK], F32).ap()
    oh_sb = nc.alloc_sbuf_tensor("oh", [128, 1], F16).ap()
    of_sb = nc.alloc_sbuf_tensor("of", [128, 1], F32).ap()
    emb_sb = [nc.alloc_sbuf_tensor(f"emb{b}", [3, S], F16).ap()
              for b in range(BC)]
    y_sb = [nc.alloc_sbuf_tensor(f"y{i}", [128, 2 * CH], F16).ap()
            for i in range(2)]
    p_sb = [nc.alloc_sbuf_tensor(f"p{i}", [128, NK * CH], F16).ap()
            for i in range(2)]
    q_sb = [nc.alloc_sbuf_tensor(f"q{i}", [128, 3 * CH], F16).ap()
            for i in range(3)]
    ex_w = nc.alloc_sbuf_tensor("ex_w", [128, S], F32).ap()
    aw = nc.alloc_sbuf_tensor("aw", [128, NCH], F32).ap()
    ex4 = nc.alloc_sbuf_tensor("ex4", [BC, S], F32).ap()
    o4 = nc.alloc_sbuf_tensor("o4", [BC, S], F32).ap()
    nm0 = nc.alloc_sbuf_tensor("nm0", [1, NBC], F32).ap()   # -chunk max
    sm0 = nc.alloc_sbuf_tensor("sm0", [1, NBC], F32).ap()   # chunk expsum
    mnb = nc.alloc_sbuf_tensor("mnb", [1, BC], F32).ap()    # min_c nm = -m_b
    ed0 = nc.alloc_sbuf_tensor("ed0", [1, NBC], F32).ap()   # exp(mx_c-m_b)
    w0 = nc.alloc_sbuf_tensor("w0", [1, NBC], F32).ap()
    zb = nc.alloc_sbuf_tensor("zb", [1, BC], F32).ap()
    rz = nc.alloc_sbuf_tensor("rz", [1, BC], F32).ap()
    al4 = nc.alloc_sbuf_tensor("al4", [BC, NCH], F32).ap()

    # PSUM: 4 banks A + 3 banks e = 7 of 8
    ps_a = nc.alloc_psum_tensor("psA", [128, NK * CH], F32).ap()
    ps_e = [nc.alloc_psum_tensor(f"psE{i}", [128, CH], F32).ap()
            for i in range(NEB)]

    dma_set = nc.alloc_semaphore("dma_set")  # oh, ha, emb (reduce deps)
    dma_m = nc.alloc_semaphore("dma_m")      # m_sb (PE main dep)
    dma_h = nc.alloc_semaphore("dma_h")      # ht, of (ACT/DVE deps)
    dma_ln = [nc.alloc_semaphore(f"dma_ln{k}") for k in range(NLANE)]
    dma_g = nc.alloc_semaphore("dma_g")
    dma_out = nc.alloc_semaphore("dma_out")
    pe_mm = nc.alloc_semaphore("pe_mm")      # +1 per kt MM-group (4/step)
    pe_red = nc.alloc_semaphore("pe_red")    # +1 per step e-reduce
    act_y = nc.alloc_semaphore("act_y")      # +1 per Y half (2/step)
    act_exp = nc.alloc_semaphore("act_exp")  # +1 per chunk exp
    act_ed = nc.alloc_semaphore("act_ed")    # +1 per batch ed
    act_fin = nc.alloc_semaphore("act_fin")
    dve_p = nc.alloc_semaphore("dve_p")      # +1 per step P01 mul
    dve_qt = nc.alloc_semaphore("dve_qt")    # endgame DVE Q1/Q2 self-sync
    dve_q2 = nc.alloc_semaphore("dve_q2")    # endgame DVE Q done
    dve_s = nc.alloc_semaphore("dve_s")      # +1 per stt (2/step)
    dve_sd = nc.alloc_semaphore("dve_sd")    # DVE col-0 seed self-sync
    dve_mx = nc.alloc_semaphore("dve_mx")    # +1 per chunk max
    dve_tl = nc.alloc_semaphore("dve_tl")    # +1 per batch mnb
    dve_tc = nc.alloc_semaphore("dve_tc")    # tail chain self-sync counter
    dve_al = nc.alloc_semaphore("dve_al")    # +1 alphas ready
    dve_fin = nc.alloc_semaphore("dve_fin")
    act_sd = nc.alloc_semaphore("act_sd")    # ACT col-0 seed self-sync
    pool_t2 = nc.alloc_semaphore("pool_t2")  # +2 per step (Q1, Q2)
    pool_q = nc.alloc_semaphore("pool_q")    # +1 per step Q done

    EXP = mybir.ActivationFunctionType.Exp

    with nc.Block() as blk:
        # --- SP: all DMAs ---
        @blk.sync
        def _(sync):
            # first chunk first: it gates the first matmul (m/ht/of go out
            # in parallel on ACT's HWDGE queue). Setup DMAs use
            # per-consumer-group semaphores (completions can reorder, so
            # partial-sum thresholds on one semaphore would be ambiguous).
            sync.dma_start(enc_sb[0][:], enc_c[0]).then_inc(dma_ln[0], 16)
            setup = [
                (oh_sb[:], one_h[:]),
                *[(emb_sb[b][:], emb_a[3 * b:3 * b + 3, :])
                  for b in range(BC)],
            ]
            for dst, src in setup:
                sync.dma_start(dst, src).then_inc(dma_set, 16)
            for bc in range(1, NBC):
                if bc >= NSLOT:
                    p = bc - NSLOT
                    sync.wait_ge(pe_mm, 4 * p + 4)
                    sync.wait_ge(dve_p, p + 1)
                    sync.wait_ge(dve_s, 2 * p + 2)
                if bc >= NLANE:
                    sync.wait_ge(dma_ln[bc % NLANE], 16 * (bc // NLANE))
                sync.dma_start(enc_sb[bc % NSLOT][:], enc_c[bc]) \
                    .then_inc(dma_ln[bc % NLANE], 16)
            # gather ex_w batch rows chunk-by-chunk as their exps complete
            for cc in range(NCH):
                sync.wait_ge(act_exp, 3 * NCH + cc + 1)  # exp(24+cc) done
                sync.dma_start(ex4[:, cc * CH:(cc + 1) * CH],
                               ex_w[0:128:32, cc * CH:(cc + 1) * CH]) \
                    .then_inc(dma_g, 16)
            sync.wait_ge(dve_al, 1)
            sync.dma_start(al4[:], aw[0:128:32, :]).then_inc(dma_g, 16)
            # out in pieces as rescale chunks land (ACT issues its own piece)
            sync.wait_ge(dve_fin, 1)
            sync.dma_start(out[:, 0:2 * CH], o4[:, 0:2 * CH]) \
                .then_inc(dma_out, 16)
            sync.wait_ge(dve_fin, 2)
            sync.dma_start(out[:, 2 * CH:5 * CH], o4[:, 2 * CH:5 * CH]) \
                .then_inc(dma_out, 16)
            sync.wait_ge(dma_out, 48)

        # --- PE ---
        @blk.tensor
        def _(tensor):
            def reduce(j):
                b, c, r = j // NCH, j % NCH, j % NEB
                if j == 0:
                    tensor.wait_ge(dma_set, 80)  # oh, emb
                if j >= NBC - 2:
                    tensor.wait_ge(dve_q2, j - (NBC - 2) + 1)
                else:
                    tensor.wait_ge(pool_q, j + 1)
                if j >= NEB:
                    tensor.wait_ge(act_exp, j - NEB + 1)  # WAR ps_e[r]
                nc.tensor.matmul(
                    ps_e[r][0:1, 0:CH], oh_sb[:, 0:1],
                    q_sb[j % 3][:, 2 * CH:3 * CH],
                    start=True, stop=False)
                nc.tensor.matmul(
                    ps_e[r][0:1, 0:CH], oh_sb[0:3, 0:1],
                    emb_sb[b][0:3, c * CH:(c + 1) * CH],
                    start=False, stop=True).then_inc(pe_red, 1)

            tensor.wait_ge(dma_m, 16)            # m_sb only
            for bc in range(NBC):
                slot = bc % NSLOT
                tensor.wait_ge(dma_ln[bc % NLANE], 16 * (bc // NLANE + 1))
                for kt in range(NK):
                    # per-bank WAR: wait only for the drain of THIS bank from
                    # the previous step, so late stt's don't stall early MMs
                    if bc >= 1:
                        if kt == 0:
                            tensor.wait_ge(act_y, 2 * bc)       # Y of bc-1
                        elif kt == 2:
                            tensor.wait_ge(dve_s, 2 * bc - 1)   # stt P2
                        elif kt == 3:
                            tensor.wait_ge(dve_s, 2 * bc)       # stt P3
                    for j in range(NK):
                        mm = nc.tensor.matmul(
                            ps_a[:, kt * CH:(kt + 1) * CH],
                            m_sb[:, j * H + kt * 128:j * H + (kt + 1) * 128],
                            enc_sb[slot][:, j * CW:j * CW + CH],
                            start=(j == 0), stop=(j == NK - 1))
                    mm.then_inc(pe_mm, 1)
                # lag-2 reduce: gives the P->Q chain a full extra step of
                # slack so the reduce never stalls the PE
                if bc >= 2:
                    reduce(bc - 2)
            reduce(NBC - 2)
            reduce(NBC - 1)

        # --- ACT: Y copies (h-fold, psum->sbuf fp16), chunk exp, batch ed ---
        @blk.scalar
        def _(scalar):
            def exp_op(j):
                b, c, r = j // NCH, j % NCH, j % NEB
                scalar.wait_ge(pe_red, j + 1)
                scalar.wait_ge(dve_mx, j + 1)
                nc.scalar.activation(
                    ex_w[32 * b:32 * b + 1, c * CH:(c + 1) * CH],
                    ps_e[r][0:1, 0:CH],
                    EXP, bias=nm0[0:1, j:j + 1],
                    accum_out=sm0[0:1, j:j + 1]).then_inc(act_exp, 1)

            def ed_op(b):
                scalar.wait_ge(dve_tl, b + 1)
                nc.scalar.activation(
                    ed0[0:1, NCH * b:NCH * (b + 1)],
                    nm0[0:1, NCH * b:NCH * (b + 1)],
                    EXP, bias=mnb[0:1, b:b + 1], scale=-1.0) \
                    .then_inc(act_ed, 1)

            # issue m/ht/of on ACT's own HWDGE queue, parallel to SP's chunk0
            scalar.dma_start(m_sb[:], m_p[:]).then_inc(dma_m, 16)
            scalar.dma_start(ht_sb[:], h_t[:]).then_inc(dma_h, 16)
            scalar.dma_start(of_sb[:], one_f[:]).then_inc(dma_h, 16)
            scalar.wait_ge(dma_h, 32)            # ht, of
            n_sd = 0
            for bc in range(NBC):
                b, c, par = bc // NCH, bc % NCH, bc % 2
                scalar.wait_ge(pe_mm, 4 * bc + 1)
                if bc >= 2:
                    scalar.wait_ge(dve_p, bc - 1)    # WAR y_sb[par]
                if c == 0:
                    nc.scalar.copy(ps_a[:, 0:1], of_sb[:]) \
                        .then_inc(act_sd, 1)
                    n_sd += 1
                    scalar.wait_ge(act_sd, n_sd)
                nc.scalar.mul(y_sb[par][:, 0:CH], ps_a[:, 0:CH],
                              ht_sb[:, NK * b:NK * b + 1]).then_inc(act_y, 1)
                scalar.wait_ge(pe_mm, 4 * bc + 2)
                if c == 0:
                    nc.scalar.copy(ps_a[:, CH:CH + 1], of_sb[:]) \
                        .then_inc(act_sd, 1)
                    n_sd += 1
                    scalar.wait_ge(act_sd, n_sd)
                nc.scalar.mul(y_sb[par][:, CH:2 * CH], ps_a[:, CH:2 * CH],
                              ht_sb[:, NK * b + 1:NK * b + 2]) \
                    .then_inc(act_y, 1)
                # lag-3 exp: pe_red/dve_mx for step j land during step j+2,
                # so an earlier exp would stall ACT and delay the next Y pair
                if bc >= 3:
                    exp_op(bc - 3)
                if bc % NCH == 3 and bc > NCH:
                    ed_op(bc // NCH - 1)
            exp_op(NBC - 3)
            exp_op(NBC - 2)
            exp_op(NBC - 1)
            ed_op(BC - 1)
            # rescale chunks 5-7, then ship that piece on ACT's own queue
            scalar.wait_ge(dma_g, 16 * (NCH + 1))
            for cc in range(5, NCH):
                nc.scalar.mul(o4[:, cc * CH:(cc + 1) * CH],
                              ex4[:, cc * CH:(cc + 1) * CH],
                              al4[0:BC, cc:cc + 1]).then_inc(act_fin, 1)
            scalar.wait_ge(act_fin, 3)           # own o4 writes acked
            scalar.dma_start(out[:, 5 * CH:S], o4[:, 5 * CH:S]) \
                .then_inc(dma_out, 16)

        # --- DVE: P01 mul, Q tree, chunk max, tail combine, rescale ---
        @blk.vector
        def _(vector):
            def mx_op(j):
                r = j % NEB
                vector.wait_ge(pe_red, j + 1)
                nc.vector.tensor_reduce(
                    nm0[0:1, j:j + 1], ps_e[r][0:1, 0:CH],
                    mybir.AxisListType.X, mybir.AluOpType.max,
                    negate=True).then_inc(dve_mx, 1)

            n_tc = 0

            def mnb_op(b):
                vector.wait_ge(dve_mx, NCH * (b + 1))  # own nm0 writes acked
                nc.vector.tensor_reduce(
                    mnb[0:1, b:b + 1], nm0[0:1, NCH * b:NCH * (b + 1)],
                    mybir.AxisListType.X, mybir.AluOpType.min) \
                    .then_inc(dve_tl, 1)

            def wz_op(b):
                nonlocal n_tc
                vector.wait_ge(act_ed, b + 1)
                vector.wait_ge(act_exp, NCH * (b + 1))
                nc.vector.tensor_mul(w0[0:1, NCH * b:NCH * (b + 1)],
                                     ed0[0:1, NCH * b:NCH * (b + 1)],
                                     sm0[0:1, NCH * b:NCH * (b + 1)]) \
                    .then_inc(dve_tc, 1)
                n_tc += 1
                vector.wait_ge(dve_tc, n_tc)
                nc.vector.tensor_reduce(
                    zb[0:1, b:b + 1], w0[0:1, NCH * b:NCH * (b + 1)],
                    mybir.AxisListType.X, mybir.AluOpType.add) \
                    .then_inc(dve_tc, 1)
                n_tc += 1

            n_sd = 0
            vector.wait_ge(dma_h, 32)            # ht, of
            for bc in range(NBC):
                b, c, par, slot = bc // NCH, bc % NCH, bc % 2, bc % NSLOT
                # P01 = Y01 * E01
                vector.wait_ge(act_y, 2 * bc + 2)
                if bc >= 2:
                    vector.wait_ge(pool_q, bc - 1)   # WAR p_sb[par]
                nc.vector.tensor_mul(
                    p_sb[par].rearrange("p (k s) -> p k s", k=NK)[:, 0:2, :],
                    y_sb[par].rearrange("p (k s) -> p k s", k=2)[:, :, :],
                    enc_sb[slot].rearrange("p (k w) -> p k w", k=NK)
                    [:, 0:2, 1:CW]).then_inc(dve_p, 1)
                # P2, P3 stt folds (psum)
                for kt in (2, 3):
                    vector.wait_ge(pe_mm, 4 * bc + kt + 1)
                    if c == 0:
                        nc.vector.tensor_copy(
                            ps_a[:, kt * CH:kt * CH + 1], of_sb[:]) \
                            .then_inc(dve_sd, 1)
                        n_sd += 1
                        vector.wait_ge(dve_sd, n_sd)
                    nc.vector.scalar_tensor_tensor(
                        p_sb[par][:, kt * CH:(kt + 1) * CH],
                        ps_a[:, kt * CH:(kt + 1) * CH],
                        ht_sb[:, NK * b + kt:NK * b + kt + 1],
                        enc_sb[slot][:, kt * CW + 1:kt * CW + CW],
                        mybir.AluOpType.mult, mybir.AluOpType.mult) \
                        .then_inc(dve_s, 1)
                if bc >= NBC - 2:
                    # endgame: no later MM block hides the pool Q latency, so
                    # DVE computes the last two Q's itself (fp16 2x ops)
                    g = bc - (NBC - 2)
                    vector.wait_ge(dve_p, bc + 1)        # own P01 acked
                    nc.vector.tensor_add(q_sb[bc % 3][:, 0:CH],
                                         p_sb[par][:, 0:CH],
                                         p_sb[par][:, CH:2 * CH]) \
                        .then_inc(dve_qt, 1)
                    vector.wait_ge(dve_s, 2 * bc + 2)    # own stt acked
                    nc.vector.tensor_add(q_sb[bc % 3][:, CH:2 * CH],
                                         p_sb[par][:, 2 * CH:3 * CH],
                                         p_sb[par][:, 3 * CH:4 * CH]) \
                        .then_inc(dve_qt, 1)
                    vector.wait_ge(dve_qt, 2 * g + 2)
                    nc.vector.tensor_add(q_sb[bc % 3][:, 2 * CH:3 * CH],
                                         q_sb[bc % 3][:, 0:CH],
                                         q_sb[bc % 3][:, CH:2 * CH]) \
                        .then_inc(dve_q2, 1)
                if bc >= 2:
                    mx_op(bc - 2)
                if bc % NCH == 2 and bc > NCH:
                    mnb_op(bc // NCH - 1)
                if bc % NCH == 3 and bc > NCH:
                    wz_op(bc // NCH - 1)
            mx_op(NBC - 2)
            mx_op(NBC - 1)
            mnb_op(BC - 1)
            wz_op(BC - 1)
            vector.wait_ge(dve_tc, n_tc)             # zb writes acked
            nc.vector.reciprocal(rz[0:1, 0:BC], zb[0:1, 0:BC]) \
                .then_inc(dve_tc, 1)
            n_tc += 1
            vector.wait_ge(dve_tc, n_tc)             # rz write acked
            for b in range(BC):
                op = nc.vector.tensor_scalar_mul(
                    aw[32 * b:32 * b + 1, 0:NCH],
                    ed0[0:1, NCH * b:NCH * (b + 1)], rz[0:1, b:b + 1])
            op.then_inc(dve_al, 1)
            # rescale chunks 0-4 (dve_fin: +1 after chunks 0-1, +1 after 2-4)
            vector.wait_ge(dma_g, 16 * (NCH + 1))
            for cc in range(5):
                op = nc.vector.tensor_scalar_mul(
                    o4[:, cc * CH:(cc + 1) * CH],
                    ex4[:, cc * CH:(cc + 1) * CH], al4[0:BC, cc:cc + 1])
                if cc == 1:
                    op.then_inc(dve_fin, 1)
            op.then_inc(dve_fin, 1)

        # --- POOL (gpsimd): P2,P3 stt folds, rescale chunks 6,7 ---
        @blk.gpsimd
        def _(gpsimd):
            for bc in range(NBC - 2):
                par, qar = bc % 2, bc % 3
                if bc >= 3:
                    gpsimd.wait_ge(pe_red, bc - 2)   # WAR q_sb[qar] (Q slice)
                gpsimd.wait_ge(dve_p, bc + 1)        # P01 landed
                nc.gpsimd.tensor_add(q_sb[qar][:, 0:CH],
                                     p_sb[par][:, 0:CH],
                                     p_sb[par][:, CH:2 * CH]) \
                    .then_inc(pool_t2, 1)
                gpsimd.wait_ge(dve_s, 2 * bc + 2)    # P2, P3 landed
                nc.gpsimd.tensor_add(q_sb[qar][:, CH:2 * CH],
                                     p_sb[par][:, 2 * CH:3 * CH],
                                     p_sb[par][:, 3 * CH:4 * CH]) \
                    .then_inc(pool_t2, 1)
                gpsimd.wait_ge(pool_t2, 2 * bc + 2)  # own writes acked
                nc.gpsimd.tensor_add(q_sb[qar][:, 2 * CH:3 * CH],
                                     q_sb[qar][:, 0:CH],
                                     q_sb[qar][:, CH:2 * CH]) \
                    .then_inc(pool_q, 1)

    return nc


def _shard_host(hidden, encoder_outputs, embedding, bigram_matrix, affect_matrix):
    """Per-core input maps. Layout/cast prep only (plus tiny h@affect)."""
    h = np.asarray(hidden, dtype=np.float32)[0]              # [B, H]
    enc = np.asarray(encoder_outputs, dtype=np.float32)      # [S, B, H]
    emb = np.asarray(embedding, dtype=np.float32)            # [S, B, 3]
    m = np.asarray(bigram_matrix, dtype=np.float32)
    aff = np.asarray(affect_matrix, dtype=np.float32)        # [H, 3]

    # padded fp16 enc: row 0 is the s=-1 halo for c==0 (value irrelevant;
    # psum col 0 is re-seeded on device)
    encp = np.zeros((S + 1, B, H), dtype=np.float16)
    encp[1:] = enc.astype(np.float16)

    m16 = m.astype(np.float16)
    m_p = np.ascontiguousarray(
        m16.reshape(NK, 128, H).transpose(1, 0, 2).reshape(128, NK * H))
    ha = h @ aff                                             # [B, 3]
    # affect term pre-scaled: emb_w[b, k, s] = ha[b,k] * emb[s, b, k]
    emb16 = np.ascontiguousarray(
        (emb.transpose(1, 2, 0) * ha[:, :, None]).astype(np.float16))
    one_h = np.ones((128, 1), dtype=np.float16)
    one_f = np.ones((128, 1), dtype=np.float32)

    in_maps = []
    for co in range(NCORES):
        b0 = co * BC
        # enc_c[b*NCH+c, p, k*CW+w] = encp[c*CH+w, b0+b, k*128+p]
        blocks = []
        for b in range(b0, b0 + BC):
            v = np.ascontiguousarray(encp[:, b, :])          # [S+1, H]
            w = np.lib.stride_tricks.as_strided(
                v, shape=(NCH, CW, H),
                strides=(CH * v.strides[0], v.strides[0], v.strides[1]))
            t = w.transpose(0, 2, 1).reshape(NCH, NK, 128, CW)
            blocks.append(t.transpose(0, 2, 1, 3).reshape(NCH, 128, NK * CW))
        enc_cc = np.ascontiguousarray(np.concatenate(blocks, axis=0))
        h_sl = h[b0:b0 + BC]                                 # [BC, H]
        ht = np.ascontiguousarray(
            h_sl.reshape(BC, NK, 128).transpose(2, 0, 1).reshape(128, BC * NK))
        in_maps.append({
            "enc_c": enc_cc,
            "m_p": m_p,
            "h_t": ht,
            "one_h": one_h,
            "one_f": one_f,
            "emb_a": emb16[b0:b0 + BC].reshape(3 * BC, S),
        })
    return in_maps


def kernel(hidden, encoder_outputs, embedding, bigram_matrix, affect_matrix,
           _want_results=False, _spmd_kwargs=None):
    nc = _build()
    in_maps = _shard_host(hidden, encoder_outputs, embedding,
                          bigram_matrix, affect_matrix)
    res = run_bass_kernel_spmd(nc, in_maps, core_ids=list(range(NCORES)),
                               **(_spmd_kwargs or {}))
    outp = np.empty((B, 1, S), dtype=np.float32)
    for co in range(NCORES):
        outp[co * BC:(co + 1) * BC, 0, :] = res.results[co]["out"]
    if _want_results:
        return outp, res
    return outp
